# revision 14
# baseline (speedup 1.0000x reference)
"""Trainium2 Bass kernel for nn_CombinedLoss (surface loss + Tversky loss).

The reference computes a 4D (C,D,H,W) Euclidean distance transform of the
one-hot argmax mask per batch element, but because the EDT includes the
channel axis (C=3) the distance maps collapse analytically:

  * pos_d == 1 at every pos voxel (a zero channel-neighbor always exists at
    distance 1), so the (pos_d - 1) * pos term is identically zero.
  * neg_d at channel 1 (the only channel SurfaceLoss reads, idc=[1]) is
    sqrt(min(spatial_dist^2_to_cls1, 1)) == 1 at every voxel with cls != 1.

  => dist_maps[:, 1] == (argmax_c probs != 1), exactly (verified vs scipy EDT).

So the whole loss is elementwise work + global reductions:

  surface = mean(p1 * [argmax != 1])        over B*D*H*W voxels
  tversky = 1 - (tp + 1) / (0.5*sum(p) + 0.5*sum(t) + 1),   tp = sum(p*t)

[argmax != 1] uses first-max-wins semantics: NOT(p1 > p0 AND p1 >= p2), i.e.
(p0 >= p1) OR (p2 > p1), computed as is_ge(max(p0, p2), p1) (exact up to
measure-zero float ties between p0/p2 and p1).

Sharding: voxels are flattened and split evenly across the 8 cores (the
losses are independent per voxel); each core reduces its shard to per-
partition partial sums, and the host does the final tiny reduction in f64.

Device-side split (raw Bass; this toolchain rejects instructions carrying
more than one sync-wait, so waits are standalone and per-engine ordering is
implicit): DVE computes the argmax indicator and the p*t partial sums via
scalar_tensor_tensor(bypass, mult) with accum_out; ACT sums all six planes
via activation(Copy, accum_out). One DMA per chunk loads the six planes
interleaved into one SBUF region.
"""

import numpy as np

import concourse.bass as bass
import concourse.mybir as mybir
from concourse.bass_utils import run_bass_kernel_spmd

N_CORES = 8
B, C, D, H, W = 2, 3, 64, 128, 128
N_VOX = B * D * H * W            # 2_097_152
VOX_PER_CORE = N_VOX // N_CORES  # 262_144
P = 128                          # partitions
NCH = 4                          # chunks per core
CW = VOX_PER_CORE // (P * NCH)   # 512 columns per chunk

_CACHE = {}


def _build_module():
    Alu = mybir.AluOpType
    Act = mybir.ActivationFunctionType
    f32 = mybir.dt.float32

    nc = bass.Bass()
    # per chunk: p0|p1|p2|t0|t1|t2 planes interleaved along the free dim
    x_in = nc.dram_tensor("x", [NCH, P, 6 * CW], f32, kind="ExternalInput")
    sdve_out = nc.dram_tensor("sdve", [P, NCH * 4], f32, kind="ExternalOutput")
    sact_out = nc.dram_tensor("sact", [P, NCH * 6], f32, kind="ExternalOutput")

    from contextlib import ExitStack

    with (
        ExitStack() as ctx,
        nc.sbuf_tensor([P, NCH * 6 * CW], f32) as big,
        nc.sbuf_tensor([P, CW], f32) as m,
        nc.sbuf_tensor([P, CW], f32) as ind,
        nc.sbuf_tensor([P, CW], f32) as vjunk,
        nc.sbuf_tensor([P, CW], f32) as ajunk,
        nc.sbuf_tensor([P, NCH * 4], f32) as sdve_sb,
        nc.sbuf_tensor([P, NCH * 6], f32) as sact_sb,
        nc.semaphore() as out1_sem,
        nc.semaphore() as out2_sem,
        nc.semaphore() as v_sem,
        nc.semaphore() as a_sem,
        nc.Block() as block,
    ):
        # one sem per chunk DMA: +16 updates on one sem from concurrent DMA
        # queues have no completion-order guarantee (CoreSim flags it)
        in_sems = [
            ctx.enter_context(nc.semaphore(f"in_sem{i}")) for i in range(NCH)
        ]
        def pl(ch, c):
            return big[:, (ch * 6 + c) * CW : (ch * 6 + c + 1) * CW]

        @block.sync
        def _(sync):
            for ch in range(NCH):
                sync.dma_start(
                    big[:, ch * 6 * CW : (ch + 1) * 6 * CW], x_in[ch]
                ).then_inc(in_sems[ch], 16)
            sync.wait_ge(v_sem, NCH * 6)
            sync.dma_start(sdve_out[:], sdve_sb[:]).then_inc(out1_sem, 16)
            sync.wait_ge(a_sem, NCH * 6)
            sync.dma_start(sact_out[:], sact_sb[:]).then_inc(out2_sem, 16)
            sync.wait_ge(out1_sem, 16)
            sync.wait_ge(out2_sem, 16)

        # Per-engine sem chains: engines execute in order on HW (per-op pipe
        # drain), but CoreSim's race detector wants explicit ordering for the
        # same-engine RAW (m, ind) and WAW (shared junk dumps). Each op incs
        # its engine sem; op j waits >= j (already satisfied at runtime).
        @block.vector
        def _(vector):
            j = 0
            for ch in range(NCH):
                vector.wait_ge(in_sems[ch], 16)
                if j:
                    vector.wait_ge(v_sem, j)
                vector.tensor_tensor(
                    m[:], pl(ch, 0), pl(ch, 2), Alu.max
                ).then_inc(v_sem, 1)
                j += 1
                vector.wait_ge(v_sem, j)
                vector.tensor_tensor(
                    ind[:], m[:], pl(ch, 1), Alu.is_ge
                ).then_inc(v_sem, 1)
                j += 1
                vector.wait_ge(v_sem, j)
                vector.scalar_tensor_tensor(
                    vjunk[:], pl(ch, 1), 0.0, ind[:], Alu.bypass, Alu.mult,
                    accum_out=sdve_sb[:, ch * 4 : ch * 4 + 1],
                ).then_inc(v_sem, 1)
                j += 1
                for c in range(C):
                    vector.wait_ge(v_sem, j)
                    vector.scalar_tensor_tensor(
                        vjunk[:], pl(ch, c), 0.0, pl(ch, 3 + c),
                        Alu.bypass, Alu.mult,
                        accum_out=sdve_sb[:, ch * 4 + 1 + c : ch * 4 + 2 + c],
                    ).then_inc(v_sem, 1)
                    j += 1

        @block.scalar
        def _(scalar):
            j = 0
            for ch in range(NCH):
                scalar.wait_ge(in_sems[ch], 16)
                for c in range(6):
                    if j:
                        scalar.wait_ge(a_sem, j)
                    scalar.activation(
                        ajunk[:], pl(ch, c), Act.Copy,
                        accum_out=sact_sb[:, ch * 6 + c : ch * 6 + c + 1],
                    ).then_inc(a_sem, 1)
                    j += 1

    return nc


def _shard(probs, target):
    """[B,C,D,H,W] f32 x2 -> per-core [NCH, P, 6*CW] contiguous arrays."""
    pf = np.ascontiguousarray(probs.transpose(1, 0, 2, 3, 4)).reshape(C, N_VOX)
    tf = np.ascontiguousarray(target.transpose(1, 0, 2, 3, 4)).reshape(C, N_VOX)
    out = []
    for k in range(N_CORES):
        sl = slice(k * VOX_PER_CORE, (k + 1) * VOX_PER_CORE)
        both = np.concatenate([pf[:, sl], tf[:, sl]])        # [6, P*NCH*CW]
        both = both.reshape(6, P, NCH, CW).transpose(2, 1, 0, 3)  # [NCH,P,6,CW]
        out.append(np.ascontiguousarray(both).reshape(NCH, P, 6 * CW))
    return out


def _finalize(results):
    s1 = tp = spt = 0.0
    for r in results:
        sdve = r["sdve"].astype(np.float64).reshape(P, NCH, 4)
        sact = r["sact"].astype(np.float64)
        s1 += sdve[:, :, 0].sum()
        tp += sdve[:, :, 1:4].sum()
        spt += sact.sum()
    surface = s1 / float(N_VOX)
    tversky = 1.0 - (tp + 1.0) / (0.5 * spt + 1.0)
    return np.float32(surface + tversky)


def kernel(probs: np.ndarray, target: np.ndarray) -> np.ndarray:
    probs = np.asarray(probs, dtype=np.float32)
    target = np.asarray(target, dtype=np.float32)

    if "nc" not in _CACHE:
        _CACHE["nc"] = _build_module()
    nc = _CACHE["nc"]

    xs = _shard(probs, target)
    in_maps = [{"x": xs[k]} for k in range(N_CORES)]
    res = run_bass_kernel_spmd(nc, in_maps, core_ids=list(range(N_CORES)))
    return _finalize(res.results)


# revision 17
# speedup vs baseline: 1.4829x; 1.4829x over previous
"""Trainium2 Bass kernel for nn_CombinedLoss (surface loss + Tversky loss).

The reference computes a 4D (C,D,H,W) Euclidean distance transform of the
one-hot argmax mask per batch element, but because the EDT includes the
channel axis (C=3) the distance maps collapse analytically:

  * pos_d == 1 at every pos voxel (a zero channel-neighbor always exists at
    distance 1), so the (pos_d - 1) * pos term is identically zero.
  * neg_d at channel 1 (the only channel SurfaceLoss reads, idc=[1]) is
    sqrt(min(spatial_dist^2_to_cls1, 1)) == 1 at every voxel with cls != 1.

  => dist_maps[:, 1] == (argmax_c probs != 1), exactly (verified vs scipy EDT).

So the whole loss is elementwise work + global reductions:

  surface = mean(p1 * [argmax != 1])        over B*D*H*W voxels
  tversky = 1 - (tp + 1) / (0.5*(sum(p)+sum(t)) + 1),   tp = sum(p*t)

Inputs are shipped to the device as bf16. [argmax != 1] = 1[max(p0,p2) >= p1]
would pick up a one-sided bias from bf16 ties, so ties count 1/2:
ind = 0.5*(is_ge + is_gt), giving ~3e-6 total relative error (validated on
the exact reference inputs on host).

Work split per core (voxels are flattened and split evenly across 8 cores;
host does the final tiny reduction in f64):
  * DVE: m=max(p0,p2); a=is_ge(m,p1); b=is_gt(m,p1); accumulate p1*a, p1*b
    via scalar_tensor_tensor(bypass,mult,accum_out).
  * PE:  tp via the diagonal trick (psa[128,129] += p_tile^T @ [t_tile|ones]
    over all channel/voxel tiles: diag = p*t partial sums, col 128 = sum(p));
    sum(t) via ones-column stationary streaming the t blocks (psb[1,387],
    ones columns baked into the t layout are subtracted on host).
Raw Bass with standalone waits (this toolchain rejects instructions carrying
more than one attached sync-wait).
"""

import numpy as np
import ml_dtypes

import concourse.bass as bass
import concourse.mybir as mybir
from concourse.bass_utils import run_bass_kernel_spmd

N_CORES = 8
B, C, D, H, W = 2, 3, 64, 128, 128
N_VOX = B * D * H * W            # 2_097_152
VOX_PER_CORE = N_VOX // N_CORES  # 262_144
P = 128                          # partitions
NCH = 4                          # chunks per core
CW = VOX_PER_CORE // (P * NCH)   # 512 columns per chunk
TPC = CW // P                    # 4 PE tiles per chunk per channel
PW = C * CW                      # 1536 p-columns per chunk
TW = C * (CW + TPC)              # 1548 t-columns per chunk (ones baked in)
N_ONES = C * NCH * TPC * P       # total baked-ones contribution to psb: 6144

_CACHE = {}


def _build_module():
    from contextlib import ExitStack

    Alu = mybir.AluOpType
    f32 = mybir.dt.float32
    bf16 = mybir.dt.bfloat16

    nc = bass.Bass()
    p_in = nc.dram_tensor("p", [NCH, P, PW], bf16, kind="ExternalInput")
    t_in = nc.dram_tensor("t", [NCH, P, TW], bf16, kind="ExternalInput")
    s1_out = nc.dram_tensor("s1", [P, NCH * 2], f32, kind="ExternalOutput")
    psa_out = nc.dram_tensor("psa", [P, CW // TPC + 1], f32, kind="ExternalOutput")
    psb_out = nc.dram_tensor("psb", [1, TW // TPC], f32, kind="ExternalOutput")

    with (
        ExitStack() as ctx,
        nc.sbuf_tensor([P, NCH * PW], bf16) as p_sb,
        nc.sbuf_tensor([P, NCH * TW], bf16) as t_sb,
        nc.sbuf_tensor([P, CW], bf16) as m_sb,
        nc.sbuf_tensor([P, CW], bf16) as a_sb,
        nc.sbuf_tensor([P, CW], bf16) as b_sb,
        nc.sbuf_tensor([P, CW], bf16) as vj_a,
        nc.sbuf_tensor([P, CW], bf16) as vj_b,
        nc.sbuf_tensor([P, NCH * 2], f32) as s1_sb,
        nc.sbuf_tensor([P, 129], f32) as psa_sb,
        nc.sbuf_tensor([1, 387], f32) as psb_sb,
        nc.psum_tensor([P, 129], f32) as psa,
        nc.psum_tensor([1, 387], f32) as psb,
        nc.semaphore() as v_sem,
        nc.semaphore() as pe_sem,
        nc.semaphore() as c_sem,
        nc.semaphore() as o1_sem,
        nc.semaphore() as o2_sem,
        nc.semaphore() as o3_sem,
        nc.Block() as block,
    ):
        p_sems = [ctx.enter_context(nc.semaphore(f"p_sem{i}")) for i in range(NCH)]
        t_sems = [ctx.enter_context(nc.semaphore(f"t_sem{i}")) for i in range(NCH)]

        def pp(ch, c):
            return p_sb[:, ch * PW + c * CW : ch * PW + (c + 1) * CW]

        def ptile(ch, c, i):
            off = ch * PW + c * CW + i * P
            return p_sb[:, off : off + P]

        def tblock(ch, c, i):
            off = ch * TW + c * (CW + TPC) + i * (P + 1)
            return t_sb[:, off : off + P + 1]

        @block.sync
        def _(sync):
            for ch in range(NCH):
                sync.dma_start(
                    p_sb[:, ch * PW : (ch + 1) * PW], p_in[ch]
                ).then_inc(p_sems[ch], 16)
                sync.dma_start(
                    t_sb[:, ch * TW : (ch + 1) * TW], t_in[ch]
                ).then_inc(t_sems[ch], 16)
            sync.wait_ge(v_sem, NCH * 5)
            sync.dma_start(s1_out[:], s1_sb[:]).then_inc(o1_sem, 16)
            sync.wait_ge(c_sem, 2)
            sync.dma_start(psa_out[:], psa_sb[:]).then_inc(o2_sem, 16)
            sync.dma_start(psb_out[:], psb_sb[:]).then_inc(o3_sem, 16)
            sync.wait_ge(o1_sem, 16)
            sync.wait_ge(o2_sem, 16)
            sync.wait_ge(o3_sem, 16)

        @block.vector
        def _(vector):
            for ch in range(NCH):
                vector.wait_ge(p_sems[ch], 16)
                if ch:
                    vector.wait_ge(v_sem, 5 * ch)
                vector.tensor_tensor(
                    m_sb[:], pp(ch, 0), pp(ch, 2), Alu.max
                ).then_inc(v_sem, 1)
                vector.wait_ge(v_sem, 5 * ch + 1)
                vector.tensor_tensor(
                    a_sb[:], m_sb[:], pp(ch, 1), Alu.is_ge
                ).then_inc(v_sem, 1)
                vector.tensor_tensor(
                    b_sb[:], m_sb[:], pp(ch, 1), Alu.is_gt
                ).then_inc(v_sem, 1)
                vector.wait_ge(v_sem, 5 * ch + 2)
                vector.scalar_tensor_tensor(
                    vj_a[:], pp(ch, 1), 0.0, a_sb[:], Alu.bypass, Alu.mult,
                    accum_out=s1_sb[:, ch * 2 : ch * 2 + 1],
                ).then_inc(v_sem, 1)
                vector.wait_ge(v_sem, 5 * ch + 3)
                vector.scalar_tensor_tensor(
                    vj_b[:], pp(ch, 1), 0.0, b_sb[:], Alu.bypass, Alu.mult,
                    accum_out=s1_sb[:, ch * 2 + 1 : ch * 2 + 2],
                ).then_inc(v_sem, 1)
            # PSUM -> SBUF copies once PE is done
            vector.wait_ge(pe_sem, 2)
            vector.tensor_copy(psa_sb[:], psa[:]).then_inc(c_sem, 1)
            vector.tensor_copy(psb_sb[:], psb[:]).then_inc(c_sem, 1)

        @block.tensor
        def _(tensor):
            n_tp = NCH * C * TPC          # 48 tp matmuls
            n_st = NCH * 4                # 16 sum(t) matmuls (387 cols each)
            k_tp = k_st = 0
            ones_col = t_sb[:, P : P + 1]  # any baked ones column
            for ch in range(NCH):
                tensor.wait_ge(p_sems[ch], 16)
                tensor.wait_ge(t_sems[ch], 16)
                for c in range(C):
                    for i in range(TPC):
                        mm = nc.tensor.matmul(
                            psa[:],
                            ptile(ch, c, i),
                            tblock(ch, c, i),
                            start=(k_tp == 0),
                            stop=(k_tp == n_tp - 1),
                        )
                        if k_tp == n_tp - 1:
                            mm.then_inc(pe_sem, 1)
                        k_tp += 1
                for q in range(4):
                    off = ch * TW + q * 387
                    mm = nc.tensor.matmul(
                        psb[:],
                        ones_col,
                        t_sb[:, off : off + 387],
                        start=(k_st == 0),
                        stop=(k_st == n_st - 1),
                    )
                    if k_st == n_st - 1:
                        mm.then_inc(pe_sem, 1)
                    k_st += 1

    return nc


def _shard(probs, target):
    """f32 [B,C,D,H,W] x2 -> per-core bf16 arrays:
    p [NCH, P, C*CW] and t [NCH, P, C*(CW+TPC)] (ones columns baked in)."""
    pf = np.ascontiguousarray(probs.transpose(1, 0, 2, 3, 4)).reshape(C, N_VOX)
    tf = np.ascontiguousarray(target.transpose(1, 0, 2, 3, 4)).reshape(C, N_VOX)
    out = []
    for k in range(N_CORES):
        sl = slice(k * VOX_PER_CORE, (k + 1) * VOX_PER_CORE)
        pk = pf[:, sl].reshape(C, P, NCH, CW).transpose(2, 1, 0, 3)
        pk = np.ascontiguousarray(pk).astype(ml_dtypes.bfloat16)
        tk4 = tf[:, sl].reshape(C, P, NCH, TPC, P).transpose(2, 1, 0, 3, 4)
        tk = np.ones((NCH, P, C, TPC, P + 1), dtype=ml_dtypes.bfloat16)
        tk[..., :P] = tk4.astype(ml_dtypes.bfloat16)
        out.append(
            (
                pk.reshape(NCH, P, PW),
                np.ascontiguousarray(tk.reshape(NCH, P, TW)),
            )
        )
    return out


def _finalize(results):
    s1 = tp = sp = st = 0.0
    for r in results:
        s1 += r["s1"].astype(np.float64).sum()
        psa = r["psa"].astype(np.float64)
        tp += np.diag(psa[:, :P]).sum()
        sp += psa[:, P].sum()
        st += r["psb"].astype(np.float64).sum() - N_ONES
    surface = 0.5 * s1 / float(N_VOX)
    tversky = 1.0 - (tp + 1.0) / (0.5 * (sp + st) + 1.0)
    return np.float32(surface + tversky)


def kernel(probs: np.ndarray, target: np.ndarray) -> np.ndarray:
    probs = np.asarray(probs, dtype=np.float32)
    target = np.asarray(target, dtype=np.float32)

    if "nc" not in _CACHE:
        _CACHE["nc"] = _build_module()
    nc = _CACHE["nc"]

    shards = _shard(probs, target)
    in_maps = [{"p": p, "t": t} for p, t in shards]
    res = run_bass_kernel_spmd(nc, in_maps, core_ids=list(range(N_CORES)))
    return _finalize(res.results)
